# revision 33
# baseline (speedup 1.0000x reference)
"""Trainium2 Bass kernel for a single DeBERTa-style attention head (v7).

Problem shapes (hardcoded):
  B=8, S=2048, E=768(n_embed), H=64(head)
  q = I @ Wq + bq ; k = x @ Wk + bk ; v = x @ Wv + bv
  w = (q @ k^T) / sqrt(E) ; w = where(mask==0, -1e9, w)
  scores = softmax(w, axis=-1) ; out = scores @ v

Sharding: data-parallel over batch B across the 8 NeuronCores (one batch
element per core, identical SPMD program).

Budget per core (measured): exp chain on ACT = 32 x [128,1024] windows
~35us; PE ~38us (projections 15 + scores 7 + ctx 14 + transposes);
DVE ~32us (2x-rate mask mults + PSUM drains); GpSimd ~28us (1x mults +
dup DMAs); DMA ~34us at ~350GB/s for 11.3MB.  All five must overlap:
  * Inputs host-packed bf16 partition-major (6-16KB descriptors; small
    descriptors cap DMA at ~250GB/s).  One sync-HWDGE FIFO carries
    everything in consumption order; the mask rides as plain uint8 (a
    cast-DMA would double its fabric bytes) and multiplies at 1x on
    DVE/GpSimd.
  * Windows run r-major in super-pairs so prerequisites arrive
    progressively; scores (K=64) are row-tiled pairs (even chunks' kT on
    partitions 0-63, odd on 64-127, qT duplicated onto both halves via
    gpsimd SBUF->SBUF DMAs - the sync/scalar rings would head-block).
  * v is projected Wv-stationary (24 wide matmuls instead of 96
    LDW-bound ones) into vT[64,S], then PE-transposed per chunk into the
    vA[k,h] layout the ctx matmul streams; vA carries a ones column so
    ctx row 64 accumulates the softmax denominator.
  * ctx matmuls are emitted ~10 windows behind their scores: engine
    queues are FIFO at runtime, and an early-emitted ctx op would
    head-block PE on the ctx accumulator's PSUM-bank WAR (the banks are
    reused from the projection pool, whose last user lands with XTb3).
  * bk dropped (softmax shift-invariant), bv applied on host, bq folded
    into the q PSUM->SBUF copy; device returns unnormalized context^T +
    denominator row ([65,S] fp32), host divides.
  * Warm-up matmuls on the weight pack keep the PE HAM clock at 2.4GHz
    through the DMA head; a dummy exp preloads the ACT table set.
"""

import math
from contextlib import ExitStack

import numpy as np

import concourse.bass as bass
import concourse.tile as tile
import concourse.mybir as mybir
from concourse import bacc
from concourse.bass_utils import run_bass_kernel_spmd

B, S, E, H = 8, 2048, 768, 64
N_CORES = 8
SC = S // 128   # 16 seq chunks
EC = E // 128   # 6 embed chunks
SCALE = 1.0 / math.sqrt(E)
CTX_LAG = 12

F32 = mybir.dt.float32
BF16 = mybir.dt.bfloat16
AF = mybir.ActivationFunctionType
ALU = mybir.AluOpType

_cache = {}


def _build_program():
    nc = bacc.Bacc("TRN2", target_bir_lowering=False, debug=False)

    dIT = nc.dram_tensor("IT", [2, 128, EC, 1024], BF16, kind="ExternalInput")
    dXT = nc.dram_tensor("XT", [4, 128, EC, 512], BF16, kind="ExternalInput")
    dmT = nc.dram_tensor("maskT", [128, SC, S], BF16, kind="ExternalInput")
    dW = nc.dram_tensor("Wpack", [E, 3 * H], BF16, kind="ExternalInput")
    dbq = nc.dram_tensor("bq2", [128, 1], F32, kind="ExternalInput")
    dId = nc.dram_tensor("ident", [64, 64], BF16, kind="ExternalInput")
    dout = nc.dram_tensor("out", [H + 1, S], F32, kind="ExternalOutput")

    with tile.TileContext(nc) as tc, ExitStack() as ctx:
        singles = ctx.enter_context(tc.tile_pool(name="singles", bufs=1))

        IT = singles.tile([128, 2, EC, 1024], BF16, tag="IT")
        XT = singles.tile([128, 4, EC, 512], BF16, tag="XT")
        w_all = singles.tile([128, EC, 3 * H], BF16, tag="Wpack")
        bq2 = singles.tile([128, 1], F32, tag="bq2")
        ident = singles.tile([64, 64], BF16, tag="ident")
        qT2 = singles.tile([128, S], BF16, tag="qT2")
        kT2 = singles.tile([128, S // 2], BF16, tag="kT2")
        vT = singles.tile([64, S], BF16, tag="vT")
        vA = singles.tile([128, SC, 66], BF16, tag="vA")
        out_sb = singles.tile([H + 1, S], F32, tag="out_sb")

        # dummy exp triggers the ACT table load (~2.7us) during the DMA head
        warm_i = singles.tile([1, 16], F32, tag="warm_i")
        warm_o = singles.tile([1, 16], BF16, tag="warm_o")
        nc.vector.memset(warm_i, 0.0)
        nc.scalar.activation(warm_o, warm_i, AF.Exp)

        nc.vector.memset(vA[:, :, 64:65], 1.0)

        # ---- input DMAs: one sync HWDGE FIFO in consumption order ----
        mpool = ctx.enter_context(tc.tile_pool(name="mpool", bufs=4))
        m_groups = [
            mpool.tile([128, 4, S], BF16, tag="m", name=f"mg{g}")
            for g in range(4)
        ]
        nc.sync.dma_start(out=IT[:, 0], in_=dIT.ap()[0])
        nc.sync.dma_start(out=XT[:, 0], in_=dXT.ap()[0])
        nc.sync.dma_start(
            out=w_all, in_=dW.ap().rearrange("(ec p) h -> p ec h", p=128)
        )
        nc.sync.dma_start(out=bq2, in_=dbq.ap())
        nc.sync.dma_start(out=ident, in_=dId.ap())
        nc.sync.dma_start(out=IT[:, 1], in_=dIT.ap()[1])
        nc.sync.dma_start(out=XT[:, 1], in_=dXT.ap()[1])
        nc.sync.dma_start(out=m_groups[0], in_=dmT.ap()[:, 0:4, :])
        nc.sync.dma_start(out=XT[:, 2], in_=dXT.ap()[2])
        nc.sync.dma_start(out=m_groups[1], in_=dmT.ap()[:, 4:8, :])
        nc.sync.dma_start(out=XT[:, 3], in_=dXT.ap()[3])
        nc.sync.dma_start(out=m_groups[2], in_=dmT.ap()[:, 8:12, :])
        nc.sync.dma_start(out=m_groups[3], in_=dmT.ap()[:, 12:16, :])

        def m_tile(c):
            return m_groups[c // 4][:, c % 4, :]

        # ---- PSUM pools: wpool 4 banks + psproj 3; the ctx accumulator
        # (4 banks) later overlaps psproj's range once it drains.
        wpool = ctx.enter_context(tc.tile_pool(name="wpool", bufs=2, space="PSUM"))
        proj_scope = ExitStack()
        psproj = proj_scope.enter_context(
            tc.tile_pool(name="psproj", bufs=1, space="PSUM")
        )

        # PE warm-up on a memset tile (no DMA dependency): the HAM clock
        # needs ~3.4us of sustained matmul activity to unthrottle, so by
        # the time ITa lands the projections run at 2.4GHz.
        wsrc = singles.tile([128, 512], BF16, tag="wsrc")
        nc.vector.memset(wsrc, 0.0)
        psw = psproj.tile([128, 1024], F32, tag="pkv",
                          name="psw")[0:64, 0:512]
        for ei in range(9):
            nc.tensor.matmul(
                psw,
                lhsT=wsrc[:, 0:64],
                rhs=wsrc,
                start=(ei == 0),
                stop=(ei == 8),
            )

        def emit_q(half):
            # two col-tiles with the same weights and stream write the
            # projection onto both partition halves at once (separate
            # PSUM banks, so each start=True zeroes only its own bank)
            for blk in range(2):
                psq = psproj.tile([128, 1024], F32, tag="pkv", name="psq")
                for ei in range(EC):
                    for half_out, osl in ((0, slice(0, 64)), (64, slice(64, 128))):
                        nc.tensor.matmul(
                            psq[osl, half_out * 8:half_out * 8 + 512],
                            lhsT=w_all[:, ei, 0:H],
                            rhs=IT[:, half, ei, blk * 512:(blk + 1) * 512],
                            start=(ei == 0),
                            stop=(ei == EC - 1),
                        )
                sl = slice(half * 1024 + blk * 512, half * 1024 + (blk + 1) * 512)
                nc.vector.tensor_scalar(
                    qT2[0:64, sl], psq[0:64, 0:512], bq2[0:64], None, ALU.add)
                nc.vector.tensor_scalar(
                    qT2[64:128, sl], psq[64:128, 512:1024], bq2[64:128], None,
                    ALU.add)

        def emit_k(blk):
            # even seq-chunks project onto partitions 0-63, odd onto
            # 64-127 (two col-tiles, strided streams, separate banks) -
            # exactly the row-tiled score layout, no shuffle DMA needed
            psk = psproj.tile([128, 1024], F32, tag="pkv", name="psk")
            xr = XT[:, blk].rearrange("p ec (four c) -> p ec four c", c=128)
            for ei in range(EC):
                nc.tensor.matmul(
                    psk[0:64, 0:256],
                    lhsT=w_all[:, ei, H:2 * H],
                    rhs=xr[:, ei, 0::2, :],
                    start=(ei == 0),
                    stop=(ei == EC - 1),
                )
                nc.tensor.matmul(
                    psk[64:128, 512:768],
                    lhsT=w_all[:, ei, H:2 * H],
                    rhs=xr[:, ei, 1::2, :],
                    start=(ei == 0),
                    stop=(ei == EC - 1),
                )
            ksl = slice(blk * 256, (blk + 1) * 256)
            nc.vector.tensor_copy(kT2[0:64, ksl], psk[0:64, 0:256])
            nc.vector.tensor_copy(kT2[64:128, ksl], psk[64:128, 512:768])

        def emit_vproj(blk):
            sl = slice(blk * 512, (blk + 1) * 512)
            psv = psproj.tile([128, 1024], F32, tag="pkv",
                              name="psv")[0:64, 0:512]
            for ei in range(EC):
                nc.tensor.matmul(
                    psv,
                    lhsT=w_all[:, ei, 2 * H:3 * H],
                    rhs=XT[:, blk, ei, :],
                    start=(ei == 0),
                    stop=(ei == EC - 1),
                )
            nc.vector.tensor_copy(vT[:, sl], psv)

        def emit_vchunk(c):
            # vT[64, c-chunk] --PE transpose--> [128, 64] --> vA[k, h]
            pst = psproj.tile([128, H], BF16, tag="pt", name="pst", bufs=2)
            nc.tensor.transpose(pst, vT[:, c * 128:(c + 1) * 128], ident)
            nc.vector.tensor_copy(vA[:, c, 0:H], pst)

        # ---- score / softmax / ctx pipeline ----
        epool = ctx.enter_context(tc.tile_pool(name="epool", bufs=8))
        spool = ctx.enter_context(tc.tile_pool(name="spool", bufs=14))

        holder = {"widx": 0}
        pending_ctx = []

        def emit_score(h, r, j, c):
            wt = wpool.tile([128, 1024], F32, tag="w", name="wt")
            for qb in range(2):
                col = h * 1024 + qb * 512
                if j == 0:
                    lhsT = kT2[0:64, r * 128:(r + 1) * 128]
                    rhs = qT2[0:64, col:col + 512]
                else:
                    lhsT = kT2[64:128, r * 128:(r + 1) * 128]
                    rhs = qT2[64:128, col:col + 512]
                nc.tensor.matmul(
                    wt[:, qb * 512:(qb + 1) * 512],
                    lhsT=lhsT, rhs=rhs, start=True, stop=True,
                )
            et = epool.tile([128, 1024], BF16, tag="e", name="et")
            nc.scalar.activation(et, wt, AF.Exp, scale=SCALE)
            st = spool.tile([128, 1024], BF16, tag="s", name="st")
            widx = holder["widx"]
            holder["widx"] += 1
            # middle windows' mask-multiplies run on the otherwise-idle
            # GpSimd (~2us each at 1x); DVE (~1.2us) takes head and tail
            msl = m_tile(c)[:, h * 1024:(h + 1) * 1024]
            nc.vector.tensor_tensor(st, et, msl, ALU.mult)
            pending_ctx.append((h, r, j, c, st))

        def emit_ctx(n):
            ctxall = holder["ctx"]
            for _ in range(min(n, len(pending_ctx))):
                h, r, j, c, st = pending_ctx.pop(0)
                for qb in range(2):
                    col = h * 1024 + qb * 512
                    nc.tensor.matmul(
                        ctxall[0:H + 1, col:col + 512],
                        lhsT=vA[:, c, 0:H + 1],
                        rhs=st[:, qb * 512:(qb + 1) * 512],
                        start=(r == 0 and j == 0),
                        stop=(r == SC // 2 - 1 and j == 1),
                    )

        emit_q(0)
        emit_k(0)
        emit_vproj(0)
        for c in range(4):
            emit_vchunk(c)

        # window order: super-pairs of chunk pairs, both q-halves per pair.
        # The remaining projection blocks are emitted at the window index
        # where PE naturally reaches them just as their XT block lands; the
        # ctx matmuls trail CTX_LAG windows so they never head-block PE on
        # the ctx accumulator's PSUM-bank WAR.
        windows = []
        for R in range(0, SC // 2, 2):
            for h in range(2):
                for r in (R, R + 1):
                    for j in range(2):
                        windows.append((h, r, j, 2 * r + j))

        def emit_blk(blk):
            emit_k(blk)
            emit_vproj(blk)
            for c in range(4 * blk, 4 * blk + 4):
                emit_vchunk(c)

        for w, (h, r, j, c) in enumerate(windows):
            if w == 2:
                emit_q(1)
            elif w == 4:
                emit_blk(1)
            elif w == 8:
                emit_blk(2)
            elif w == 12:
                emit_blk(3)
                proj_scope.close()
                psctx = ctx.enter_context(
                    tc.tile_pool(name="psctx", bufs=1, space="PSUM"))
                holder["ctx"] = psctx.tile([128, S], F32, tag="ctxall",
                                           name="ctxall")
            emit_score(h, r, j, c)
            if holder["widx"] > CTX_LAG:
                emit_ctx(2)
        emit_ctx(len(pending_ctx))

        for h in range(2):
            hs = slice(h * 1024, (h + 1) * 1024)
            nc.scalar.copy(out_sb[:, hs], holder["ctx"][0:H + 1, hs])
            nc.sync.dma_start(out=dout.ap()[:, hs], in_=out_sb[:, hs])

    nc.compile()
    return nc


def get_program():
    if "nc" not in _cache:
        _cache["nc"] = _build_program()
    return _cache["nc"]


def make_in_maps(I, x, mask, Wq, bq, Wk, bk, Wv, bv):
    import ml_dtypes

    BF = ml_dtypes.bfloat16
    I = np.asarray(I, dtype=np.float32)
    x = np.asarray(x, dtype=np.float32)
    mask = np.asarray(mask, dtype=np.int32)
    Wpack = np.concatenate(
        [
            np.asarray(Wq, dtype=np.float32),
            np.asarray(Wk, dtype=np.float32),
            np.asarray(Wv, dtype=np.float32),
        ],
        axis=1,
    ).astype(BF)
    bq2 = np.tile(np.asarray(bq, np.float32).reshape(H, 1), (2, 1))
    ident = np.eye(64, dtype=np.float32).astype(BF)

    def pack_blocks(a, nblk):
        w = S // nblk
        t = np.ascontiguousarray(a.T).astype(BF).reshape(EC, 128, nblk, w)
        return np.ascontiguousarray(t.transpose(2, 1, 0, 3))

    def pack_mask(m):
        return np.ascontiguousarray(
            m.T.astype(BF).reshape(SC, 128, S).transpose(1, 0, 2)
        )

    return [
        {
            "IT": pack_blocks(I[b], 2),
            "XT": pack_blocks(x[b], 4),
            "maskT": pack_mask(mask[b]),
            "Wpack": Wpack,
            "bq2": bq2,
            "ident": ident,
        }
        for b in range(B)
    ]


def postprocess(raw, bv):
    """raw: [65, S] f32 (64 ctx rows + denominator). Returns [S, H] f32."""
    return (raw[0:H] / raw[H:H + 1]).T + np.asarray(bv, np.float32)


def kernel(I, x, mask, Wq, bq, Wk, bk, Wv, bv):
    nc = get_program()
    in_maps = make_in_maps(I, x, mask, Wq, bq, Wk, bk, Wv, bv)
    res = run_bass_kernel_spmd(nc, in_maps, list(range(N_CORES)))
    out = np.stack(
        [postprocess(res.results[b]["out"], bv) for b in range(B)], axis=0
    )
    return out.astype(np.float32)


# revision 34
# speedup vs baseline: 1.2509x; 1.2509x over previous
"""Trainium2 Bass kernel for a single DeBERTa-style attention head (v7).

Problem shapes (hardcoded):
  B=8, S=2048, E=768(n_embed), H=64(head)
  q = I @ Wq + bq ; k = x @ Wk + bk ; v = x @ Wv + bv
  w = (q @ k^T) / sqrt(E) ; w = where(mask==0, -1e9, w)
  scores = softmax(w, axis=-1) ; out = scores @ v

Sharding: data-parallel over batch B across the 8 NeuronCores (one batch
element per core, identical SPMD program).

Budget per core (measured): exp chain on ACT = 32 x [128,1024] windows
~35us; PE ~38us (projections 15 + scores 7 + ctx 14 + transposes);
DVE ~32us (2x-rate mask mults + PSUM drains); GpSimd ~28us (1x mults +
dup DMAs); DMA ~34us at ~350GB/s for 11.3MB.  All five must overlap:
  * Inputs host-packed bf16 partition-major (6-16KB descriptors; small
    descriptors cap DMA at ~250GB/s).  One sync-HWDGE FIFO carries
    everything in consumption order; the mask rides as plain uint8 (a
    cast-DMA would double its fabric bytes) and multiplies at 1x on
    DVE/GpSimd.
  * Windows run r-major in super-pairs so prerequisites arrive
    progressively; scores (K=64) are row-tiled pairs (even chunks' kT on
    partitions 0-63, odd on 64-127, qT duplicated onto both halves via
    gpsimd SBUF->SBUF DMAs - the sync/scalar rings would head-block).
  * v is projected Wv-stationary (24 wide matmuls instead of 96
    LDW-bound ones) into vT[64,S], then PE-transposed per chunk into the
    vA[k,h] layout the ctx matmul streams; vA carries a ones column so
    ctx row 64 accumulates the softmax denominator.
  * ctx matmuls are emitted ~10 windows behind their scores: engine
    queues are FIFO at runtime, and an early-emitted ctx op would
    head-block PE on the ctx accumulator's PSUM-bank WAR (the banks are
    reused from the projection pool, whose last user lands with XTb3).
  * bk dropped (softmax shift-invariant), bv applied on host, bq folded
    into the q PSUM->SBUF copy; device returns unnormalized context^T +
    denominator row ([65,S] fp32), host divides.
  * Warm-up matmuls on the weight pack keep the PE HAM clock at 2.4GHz
    through the DMA head; a dummy exp preloads the ACT table set.
"""

import math
from contextlib import ExitStack

import numpy as np

import concourse.bass as bass
import concourse.tile as tile
import concourse.mybir as mybir
from concourse import bacc
from concourse.bass_utils import run_bass_kernel_spmd

B, S, E, H = 8, 2048, 768, 64
N_CORES = 8
SC = S // 128   # 16 seq chunks
EC = E // 128   # 6 embed chunks
SCALE = 1.0 / math.sqrt(E)
CTX_LAG = 12

F32 = mybir.dt.float32
BF16 = mybir.dt.bfloat16
AF = mybir.ActivationFunctionType
ALU = mybir.AluOpType

_cache = {}


def _build_program():
    nc = bacc.Bacc("TRN2", target_bir_lowering=False, debug=False)

    dIT = nc.dram_tensor("IT", [2, 128, EC, 1024], BF16, kind="ExternalInput")
    dXT = nc.dram_tensor("XT", [4, 128, EC, 512], BF16, kind="ExternalInput")
    dmT = nc.dram_tensor("maskT", [128, SC, S], BF16, kind="ExternalInput")
    dW = nc.dram_tensor("Wpack", [E, 3 * H], BF16, kind="ExternalInput")
    dbq = nc.dram_tensor("bq2", [128, 1], F32, kind="ExternalInput")
    dId = nc.dram_tensor("ident", [64, 64], BF16, kind="ExternalInput")
    dout = nc.dram_tensor("out", [H + 1, S], F32, kind="ExternalOutput")

    with tile.TileContext(nc) as tc, ExitStack() as ctx:
        singles = ctx.enter_context(tc.tile_pool(name="singles", bufs=1))

        IT = singles.tile([128, 2, EC, 1024], BF16, tag="IT")
        XT = singles.tile([128, 4, EC, 512], BF16, tag="XT")
        w_all = singles.tile([128, EC, 3 * H], BF16, tag="Wpack")
        bq2 = singles.tile([128, 1], F32, tag="bq2")
        ident = singles.tile([64, 64], BF16, tag="ident")
        qT2 = singles.tile([128, S], BF16, tag="qT2")
        kT2 = singles.tile([128, S // 2], BF16, tag="kT2")
        vT = singles.tile([64, S], BF16, tag="vT")
        vA = singles.tile([128, SC, 66], BF16, tag="vA")
        out_sb = singles.tile([H + 1, S], F32, tag="out_sb")

        # dummy exp triggers the ACT table load (~2.7us) during the DMA head
        warm_i = singles.tile([1, 16], F32, tag="warm_i")
        warm_o = singles.tile([1, 16], BF16, tag="warm_o")
        nc.vector.memset(warm_i, 0.0)
        nc.scalar.activation(warm_o, warm_i, AF.Exp)

        nc.vector.memset(vA[:, :, 64:65], 1.0)

        # ---- input DMAs: one sync HWDGE FIFO in consumption order ----
        mpool = ctx.enter_context(tc.tile_pool(name="mpool", bufs=4))
        m_groups = [
            mpool.tile([128, 4, S], BF16, tag="m", name=f"mg{g}")
            for g in range(4)
        ]
        nc.sync.dma_start(out=IT[:, 0], in_=dIT.ap()[0])
        nc.sync.dma_start(
            out=w_all, in_=dW.ap().rearrange("(ec p) h -> p ec h", p=128)
        )
        nc.sync.dma_start(out=bq2, in_=dbq.ap())
        nc.sync.dma_start(out=ident, in_=dId.ap())
        nc.sync.dma_start(out=XT[:, 0], in_=dXT.ap()[0])
        nc.sync.dma_start(out=IT[:, 1], in_=dIT.ap()[1])
        nc.sync.dma_start(out=XT[:, 1], in_=dXT.ap()[1])
        nc.sync.dma_start(out=m_groups[0], in_=dmT.ap()[:, 0:4, :])
        nc.sync.dma_start(out=XT[:, 2], in_=dXT.ap()[2])
        nc.sync.dma_start(out=m_groups[1], in_=dmT.ap()[:, 4:8, :])
        nc.sync.dma_start(out=XT[:, 3], in_=dXT.ap()[3])
        nc.sync.dma_start(out=m_groups[2], in_=dmT.ap()[:, 8:12, :])
        nc.sync.dma_start(out=m_groups[3], in_=dmT.ap()[:, 12:16, :])

        def m_tile(c):
            return m_groups[c // 4][:, c % 4, :]

        # ---- PSUM pools: wpool 4 banks + psproj 3; the ctx accumulator
        # (4 banks) later overlaps psproj's range once it drains.
        wpool = ctx.enter_context(tc.tile_pool(name="wpool", bufs=2, space="PSUM"))
        proj_scope = ExitStack()
        psproj = proj_scope.enter_context(
            tc.tile_pool(name="psproj", bufs=1, space="PSUM")
        )

        # PE warm-up on a memset tile (no DMA dependency): the HAM clock
        # needs ~3.4us of sustained matmul activity to unthrottle, so by
        # the time ITa lands the projections run at 2.4GHz.
        wsrc = singles.tile([128, 512], BF16, tag="wsrc")
        nc.vector.memset(wsrc, 0.0)
        psw = psproj.tile([128, 1024], F32, tag="pkv",
                          name="psw")[0:64, 0:512]
        for ei in range(14):
            nc.tensor.matmul(
                psw,
                lhsT=wsrc[:, 0:64],
                rhs=wsrc,
                start=(ei == 0),
                stop=(ei == 13),
            )

        def emit_q(half):
            # two col-tiles with the same weights and stream write the
            # projection onto both partition halves at once (separate
            # PSUM banks, so each start=True zeroes only its own bank)
            for blk in range(2):
                psq = psproj.tile([128, 1024], F32, tag="pkv", name="psq")
                for ei in range(EC):
                    for half_out, osl in ((0, slice(0, 64)), (64, slice(64, 128))):
                        nc.tensor.matmul(
                            psq[osl, half_out * 8:half_out * 8 + 512],
                            lhsT=w_all[:, ei, 0:H],
                            rhs=IT[:, half, ei, blk * 512:(blk + 1) * 512],
                            start=(ei == 0),
                            stop=(ei == EC - 1),
                        )
                sl = slice(half * 1024 + blk * 512, half * 1024 + (blk + 1) * 512)
                nc.vector.tensor_scalar(
                    qT2[0:64, sl], psq[0:64, 0:512], bq2[0:64], None, ALU.add)
                nc.vector.tensor_scalar(
                    qT2[64:128, sl], psq[64:128, 512:1024], bq2[64:128], None,
                    ALU.add)

        def emit_k(blk):
            # even seq-chunks project onto partitions 0-63, odd onto
            # 64-127 (two col-tiles, strided streams, separate banks) -
            # exactly the row-tiled score layout, no shuffle DMA needed
            psk = psproj.tile([128, 1024], F32, tag="pkv", name="psk")
            xr = XT[:, blk].rearrange("p ec (four c) -> p ec four c", c=128)
            for ei in range(EC):
                nc.tensor.matmul(
                    psk[0:64, 0:256],
                    lhsT=w_all[:, ei, H:2 * H],
                    rhs=xr[:, ei, 0::2, :],
                    start=(ei == 0),
                    stop=(ei == EC - 1),
                )
                nc.tensor.matmul(
                    psk[64:128, 512:768],
                    lhsT=w_all[:, ei, H:2 * H],
                    rhs=xr[:, ei, 1::2, :],
                    start=(ei == 0),
                    stop=(ei == EC - 1),
                )
            ksl = slice(blk * 256, (blk + 1) * 256)
            nc.vector.tensor_copy(kT2[0:64, ksl], psk[0:64, 0:256])
            nc.vector.tensor_copy(kT2[64:128, ksl], psk[64:128, 512:768])

        def emit_vproj(blk):
            sl = slice(blk * 512, (blk + 1) * 512)
            psv = psproj.tile([128, 1024], F32, tag="pkv",
                              name="psv")[0:64, 0:512]
            for ei in range(EC):
                nc.tensor.matmul(
                    psv,
                    lhsT=w_all[:, ei, 2 * H:3 * H],
                    rhs=XT[:, blk, ei, :],
                    start=(ei == 0),
                    stop=(ei == EC - 1),
                )
            nc.vector.tensor_copy(vT[:, sl], psv)

        def emit_vchunk(c):
            # vT[64, c-chunk] --PE transpose--> [128, 64] --> vA[k, h]
            pst = psproj.tile([128, H], BF16, tag="pt", name="pst", bufs=2)
            nc.tensor.transpose(pst, vT[:, c * 128:(c + 1) * 128], ident)
            nc.vector.tensor_copy(vA[:, c, 0:H], pst)

        # ---- score / softmax / ctx pipeline ----
        epool = ctx.enter_context(tc.tile_pool(name="epool", bufs=8))
        spool = ctx.enter_context(tc.tile_pool(name="spool", bufs=14))

        holder = {"widx": 0}
        pending_ctx = []

        def emit_score(h, r, j, c):
            wt = wpool.tile([128, 1024], F32, tag="w", name="wt")
            for qb in range(2):
                col = h * 1024 + qb * 512
                if j == 0:
                    lhsT = kT2[0:64, r * 128:(r + 1) * 128]
                    rhs = qT2[0:64, col:col + 512]
                else:
                    lhsT = kT2[64:128, r * 128:(r + 1) * 128]
                    rhs = qT2[64:128, col:col + 512]
                nc.tensor.matmul(
                    wt[:, qb * 512:(qb + 1) * 512],
                    lhsT=lhsT, rhs=rhs, start=True, stop=True,
                )
            et = epool.tile([128, 1024], BF16, tag="e", name="et")
            nc.scalar.activation(et, wt, AF.Exp, scale=SCALE)
            st = spool.tile([128, 1024], BF16, tag="s", name="st")
            widx = holder["widx"]
            holder["widx"] += 1
            # middle windows' mask-multiplies run on the otherwise-idle
            # GpSimd (~2us each at 1x); DVE (~1.2us) takes head and tail
            msl = m_tile(c)[:, h * 1024:(h + 1) * 1024]
            nc.vector.tensor_tensor(st, et, msl, ALU.mult)
            pending_ctx.append((h, r, j, c, st))

        def emit_ctx(n):
            ctxall = holder["ctx"]
            for _ in range(min(n, len(pending_ctx))):
                h, r, j, c, st = pending_ctx.pop(0)
                for qb in range(2):
                    col = h * 1024 + qb * 512
                    nc.tensor.matmul(
                        ctxall[0:H + 1, col:col + 512],
                        lhsT=vA[:, c, 0:H + 1],
                        rhs=st[:, qb * 512:(qb + 1) * 512],
                        start=(r == 0 and j == 0),
                        stop=(r == SC // 2 - 1 and j == 1),
                    )

        emit_q(0)
        emit_k(0)
        emit_vproj(0)
        for c in range(4):
            emit_vchunk(c)

        # window order: super-pairs of chunk pairs, both q-halves per pair.
        # The remaining projection blocks are emitted at the window index
        # where PE naturally reaches them just as their XT block lands; the
        # ctx matmuls trail CTX_LAG windows so they never head-block PE on
        # the ctx accumulator's PSUM-bank WAR.
        windows = []
        for R in range(0, SC // 2, 2):
            for h in range(2):
                for r in (R, R + 1):
                    for j in range(2):
                        windows.append((h, r, j, 2 * r + j))

        def emit_blk(blk):
            emit_k(blk)
            emit_vproj(blk)
            for c in range(4 * blk, 4 * blk + 4):
                emit_vchunk(c)

        for w, (h, r, j, c) in enumerate(windows):
            if w == 2:
                emit_q(1)
            elif w == 4:
                emit_blk(1)
            elif w == 8:
                emit_blk(2)
            elif w == 12:
                emit_blk(3)
                proj_scope.close()
                psctx = ctx.enter_context(
                    tc.tile_pool(name="psctx", bufs=1, space="PSUM"))
                holder["ctx"] = psctx.tile([128, S], F32, tag="ctxall",
                                           name="ctxall")
            emit_score(h, r, j, c)
            if holder["widx"] > CTX_LAG:
                emit_ctx(2)
        emit_ctx(len(pending_ctx))

        for h in range(2):
            hs = slice(h * 1024, (h + 1) * 1024)
            nc.scalar.copy(out_sb[:, hs], holder["ctx"][0:H + 1, hs])
            nc.sync.dma_start(out=dout.ap()[:, hs], in_=out_sb[:, hs])

    nc.compile()
    return nc


def get_program():
    if "nc" not in _cache:
        _cache["nc"] = _build_program()
    return _cache["nc"]


def make_in_maps(I, x, mask, Wq, bq, Wk, bk, Wv, bv):
    import ml_dtypes

    BF = ml_dtypes.bfloat16
    I = np.asarray(I, dtype=np.float32)
    x = np.asarray(x, dtype=np.float32)
    mask = np.asarray(mask, dtype=np.int32)
    Wpack = np.concatenate(
        [
            np.asarray(Wq, dtype=np.float32),
            np.asarray(Wk, dtype=np.float32),
            np.asarray(Wv, dtype=np.float32),
        ],
        axis=1,
    ).astype(BF)
    bq2 = np.tile(np.asarray(bq, np.float32).reshape(H, 1), (2, 1))
    ident = np.eye(64, dtype=np.float32).astype(BF)

    def pack_blocks(a, nblk):
        w = S // nblk
        t = np.ascontiguousarray(a.T).astype(BF).reshape(EC, 128, nblk, w)
        return np.ascontiguousarray(t.transpose(2, 1, 0, 3))

    def pack_mask(m):
        return np.ascontiguousarray(
            m.T.astype(BF).reshape(SC, 128, S).transpose(1, 0, 2)
        )

    return [
        {
            "IT": pack_blocks(I[b], 2),
            "XT": pack_blocks(x[b], 4),
            "maskT": pack_mask(mask[b]),
            "Wpack": Wpack,
            "bq2": bq2,
            "ident": ident,
        }
        for b in range(B)
    ]


def postprocess(raw, bv):
    """raw: [65, S] f32 (64 ctx rows + denominator). Returns [S, H] f32."""
    return (raw[0:H] / raw[H:H + 1]).T + np.asarray(bv, np.float32)


def kernel(I, x, mask, Wq, bq, Wk, bk, Wv, bv):
    nc = get_program()
    in_maps = make_in_maps(I, x, mask, Wq, bq, Wk, bk, Wv, bv)
    res = run_bass_kernel_spmd(nc, in_maps, list(range(N_CORES)))
    out = np.stack(
        [postprocess(res.results[b]["out"], bv) for b in range(B)], axis=0
    )
    return out.astype(np.float32)
